# revision 1
# baseline (speedup 1.0000x reference)
"""GCN discriminator kernel for 8 Trainium2 NeuronCores.

Strategy (edge-parallel by destination): append self-loops, bucket all
edges by destination node so core i owns destination rows
[i*6250, (i+1)*6250).  Each core:
  pass 1: degree of its own dst rows via one-hot matmuls (pad edges have
          an out-of-range local col, so their one-hot row is zero),
          s = 1/sqrt(deg), AllGather s across the 8 cores.
  build:  x' = s * x written to a local DRAM table (row r pre-scaled by
          its source normalization).
  pass 2: dma_gather x'-rows per 128-edge tile (int16 indices, so the
          table is addressed through a low [0,32768) view and a high
          view; every (dst-block, lo/hi) run is padded to whole tiles),
          one-hot(dst local col) as matmul lhsT accumulates y in PSUM;
          per dst block: y @ W + b_conv, scale rows by s_dst, sigmoid,
          and a ones-vector matmul keeps a running column sum in PSUM.
  final:  AllReduce the [1,256] partial sums, mean, dot w_lin, sigmoid.
Only the scalar output leaves the cores; the [N,256] activation is never
materialized in HBM.
"""

import sys

for _p in ("/opt/trn_rl_repo", "/root/.axon_site/_ro/trn_rl_repo"):
    if _p not in sys.path:
        sys.path.insert(0, _p)

import numpy as np

N = 50000
E = 800000
D = 256
C = 8            # cores
NS = N // C      # dst rows per core
P = 128
B = (NS + P - 1) // P          # dst blocks per core (49; last has 106 rows)
LAST_ROWS = NS - (B - 1) * P   # 106
SPLIT = 32768                  # int16 index reach of dma_gather
CHUNK = 8                      # tiles per dma_gather instruction
IDXW = P // 16                 # idx columns per tile (wrapped in 16 parts)
XT = (N + 4 * P - 1) // (4 * P)  # 4-row-group tiles for the x' build (98)

_cache = {}


def _schedule(T_seg):
    """Static tile/chunk schedule shared by host prep and program build.
    T_seg: [B][2] tiles per (block, lo/hi segment).  Returns
    (blk_of, first, last, chunks) where chunks = (j0, ntiles, seg)."""
    blk_of = []
    chunks = []
    j = 0
    for b in range(B):
        for seg in range(2):
            nt = T_seg[b][seg]
            blk_of += [b] * nt
            left = nt
            while left > 0:
                take = min(CHUNK, left)
                chunks.append((j, take, seg))
                j += take
                left -= take
    first = {}
    last = {}
    for jj, b in enumerate(blk_of):
        if b not in first:
            first[b] = jj
        last[b] = jj
    return blk_of, first, last, chunks


def _prep_host(edge_index):
    """Bucket edges (+self loops) by (core, dst block, lo/hi source row);
    pad so every core has the same per-(block,seg) tile count.  Pad slots
    get idx=0 and local col 128 (one-hot row becomes all-zero)."""
    r_all = np.concatenate([np.asarray(edge_index[0], np.int64), np.arange(N, dtype=np.int64)])
    c_all = np.concatenate([np.asarray(edge_index[1], np.int64), np.arange(N, dtype=np.int64)])
    core = c_all // NS
    rem = c_all % NS
    cl = rem % P
    seg = (r_all >= SPLIT).astype(np.int64)
    bucket = (core * B + rem // P) * 2 + seg
    order = np.argsort(bucket, kind="stable")
    counts = np.bincount(bucket, minlength=C * B * 2).reshape(C, B, 2)
    T_seg = (-(-counts // P)).max(axis=0)          # [B, 2]
    T = int(T_seg.sum())

    starts = np.zeros(C * B * 2 + 1, np.int64)
    np.cumsum(counts.reshape(-1), out=starts[1:])
    tile_off = np.zeros(B * 2 + 1, np.int64)
    np.cumsum(T_seg.reshape(-1), out=tile_off[1:])

    r_t = np.zeros((C, T * P), np.int64)               # pad idx -> row 0
    cl_p = np.full((C, T * P), P, np.float32)          # pad col -> 128
    rs = r_all[order]
    cs = cl[order]
    cflat = counts.reshape(C, -1)
    for ci in range(C):
        for bs in range(B * 2):
            cnt = int(cflat[ci, bs])
            s0 = int(starts[ci * B * 2 + bs])
            d0 = int(tile_off[bs]) * P
            r_t[ci, d0:d0 + cnt] = rs[s0:s0 + cnt] - (SPLIT if bs % 2 else 0)
            cl_p[ci, d0:d0 + cnt] = cs[s0:s0 + cnt]

    # [C, T*P] -> [C, P, T]: tile j is column j, edge slot q is partition q
    cl_p = np.ascontiguousarray(cl_p.reshape(C, T, P).transpose(0, 2, 1))
    # idx param: element i of a chunk at [i % 16, chunk_col0 + i // 16],
    # replicated 8x down the partitions.  Chunk cols are tile-aligned, so
    # tile j owns idx columns [j*IDXW, (j+1)*IDXW).
    idx16 = r_t.reshape(C, T * P // 16, 16).transpose(0, 2, 1).astype(np.int16)
    idx_p = np.ascontiguousarray(np.tile(idx16, (1, 8, 1)))  # [C, 128, T*8]

    return idx_p, cl_p, [[int(v) for v in x] for x in T_seg], T


def _build(T_seg, T, debug=False):
    from concourse import bass, bacc, mybir
    import concourse.tile as tile
    from concourse.masks import make_identity

    f32 = mybir.dt.float32
    i16 = mybir.dt.int16

    nc = bacc.Bacc(
        "TRN2",
        target_bir_lowering=False,
        debug=False,
        num_devices=C,
    )

    x_d = nc.declare_dram_parameter("x", [N, D], f32, isOutput=False)
    idx_d = nc.declare_dram_parameter("idx", [P, T * IDXW], i16, isOutput=False)
    cl_d = nc.declare_dram_parameter("cl", [P, T], f32, isOutput=False)
    W_d = nc.declare_dram_parameter("W", [D, D], f32, isOutput=False)
    bc_d = nc.declare_dram_parameter("bconv", [1, D], f32, isOutput=False)
    wl_d = nc.declare_dram_parameter("wlin", [1, D], f32, isOutput=False)
    bl_d = nc.declare_dram_parameter("blin", [1, 1], f32, isOutput=False)
    io_d = nc.declare_dram_parameter("iota", [P, P], f32, isOutput=False)
    out_d = nc.declare_dram_parameter("out", [1, 1], f32, isOutput=True)
    if debug:
        dbg_s = nc.declare_dram_parameter("dbg_s", [P, B], f32, isOutput=True)
        dbg_y = nc.declare_dram_parameter("dbg_y", [P, D], f32, isOutput=True)
        dbg_mp = nc.declare_dram_parameter("dbg_mp", [1, D], f32, isOutput=True)
        dbg_ms = nc.declare_dram_parameter("dbg_ms", [1, D], f32, isOutput=True)
        dbg_deg = nc.declare_dram_parameter("dbg_deg", [P, B], f32, isOutput=True)
        dbg_xg = nc.declare_dram_parameter("dbg_xg", [P, CHUNK, D], f32, isOutput=True)

    s_own = nc.dram_tensor("s_own", [NS, 1], f32)
    s_allg = nc.dram_tensor("s_allg", [N, 1], f32, addr_space="Shared")
    xp_d = nc.dram_tensor("xp", [N, D], f32)
    mr_in = nc.dram_tensor("mr_in", [1, D], f32)
    mr_out = nc.dram_tensor("mr_out", [1, D], f32, addr_space="Shared")

    blk_of, first, last, chunks = _schedule(T_seg)

    with tile.TileContext(nc) as tc:
        with tc.tile_pool(name="static", bufs=1) as st, \
             tc.tile_pool(name="oh", bufs=3) as ohp, \
             tc.tile_pool(name="xg", bufs=3) as xgp, \
             tc.tile_pool(name="xb", bufs=4) as xbp, \
             tc.tile_pool(name="ep", bufs=2) as epp, \
             tc.tile_pool(name="pd", bufs=2, space="PSUM") as pdp, \
             tc.tile_pool(name="py", bufs=2, space="PSUM") as pyp, \
             tc.tile_pool(name="pt", bufs=1, space="PSUM") as ptp, \
             tc.tile_pool(name="pz", bufs=1, space="PSUM") as pzp, \
             tc.tile_pool(name="pm", bufs=1, space="PSUM") as pmp:

            # ---- static loads ----
            idx_sb = st.tile([P, T * IDXW], i16, tag="idx")
            cl_sb = st.tile([P, T], f32, tag="cl")
            nc.sync.dma_start(out=idx_sb[:], in_=idx_d[:])
            nc.sync.dma_start(out=cl_sb[:], in_=cl_d[:])
            iota_sb = st.tile([P, P], f32, tag="iota")
            nc.sync.dma_start(out=iota_sb[:], in_=io_d[:])
            W0_sb = st.tile([P, D], f32, tag="w0")
            W1_sb = st.tile([P, D], f32, tag="w1")
            nc.sync.dma_start(out=W0_sb[:], in_=W_d[0:P, :])
            nc.sync.dma_start(out=W1_sb[:], in_=W_d[P:D, :])
            bc_sb = st.tile([1, D], f32, tag="bc")
            nc.sync.dma_start(out=bc_sb[:], in_=bc_d[:])
            wl_sb = st.tile([1, D], f32, tag="wl")
            nc.sync.dma_start(out=wl_sb[:], in_=wl_d[:])
            bl_sb = st.tile([1, 1], f32, tag="bl")
            nc.sync.dma_start(out=bl_sb[:], in_=bl_d[:])
            ident = st.tile([P, P], f32, tag="ident")
            make_identity(nc, ident[:])
            ones_c = st.tile([P, 1], f32, tag="onesc")
            nc.vector.memset(ones_c[:], 1.0)
            ones_r = st.tile([1, P], f32, tag="onesr")
            nc.vector.memset(ones_r[:], 1.0)
            s_all = st.tile([P, B], f32, tag="sall")
            if debug:
                deg_all = st.tile([P, B], f32, tag="degall")

            # ---- pass 1: degree -> s = 1/sqrt(deg) for own dst rows ----
            for (j0, nt, seg) in chunks:
                oh = ohp.tile([P, CHUNK, P], f32, tag="oh1")
                nc.vector.tensor_tensor(
                    out=oh[:, 0:nt, :],
                    in0=cl_sb[:, j0:j0 + nt, None].to_broadcast((P, nt, P)),
                    in1=iota_sb[:, None, :].to_broadcast((P, nt, P)),
                    op=mybir.AluOpType.is_equal,
                )
                for kk in range(nt):
                    j = j0 + kk
                    b = blk_of[j]
                    if j == first[b]:
                        deg_ps = pdp.tile([P, 1], f32, tag="deg")
                        first[b] = (j, deg_ps)
                    else:
                        deg_ps = first[b][1]
                    nc.tensor.matmul(
                        out=deg_ps[:],
                        lhsT=oh[:, kk, :],
                        rhs=ones_c[:],
                        start=(first[b][0] == j),
                        stop=(last[b] == j),
                    )
                    if last[b] == j:
                        if debug:
                            nc.vector.tensor_copy(
                                out=deg_all[:, b:b + 1], in_=deg_ps[:]
                            )
                        sq = epp.tile([P, 1], f32, tag="sq")
                        nc.scalar.activation(
                            sq[:], deg_ps[:],
                            mybir.ActivationFunctionType.Sqrt,
                            bias=0.0,
                        )
                        nc.vector.reciprocal(s_all[:, b:b + 1], sq[:])
                        rows = LAST_ROWS if b == B - 1 else P
                        nc.sync.dma_start(
                            out=s_own[b * P:b * P + rows, :],
                            in_=s_all[0:rows, b:b + 1],
                        )

            if debug:
                nc.sync.dma_start(out=dbg_s[:], in_=s_all[:])
                nc.sync.dma_start(out=dbg_deg[:], in_=deg_all[:])

            nc.gpsimd.collective_compute(
                "AllGather",
                mybir.AluOpType.bypass,
                replica_groups=[list(range(C))],
                ins=[s_own[:]],
                outs=[s_allg[:]],
            )

            # ---- build x' = s * x (512 rows per step; 80-row tail) ----
            GR = 4 * P

            def scale_rows(a, ro):
                rows = ro * P
                xv = x_d[a:a + rows, :].rearrange("(o p) d -> p o d", p=P)
                xt_sb = xbp.tile([P, 4, D], f32, tag="xt")
                nc.sync.dma_start(out=xt_sb[:, 0:ro, :], in_=xv)
                sv = s_allg[a:a + rows, :].rearrange("(o p) one -> p (o one)", p=P)
                st_sb = xbp.tile([P, 4], f32, tag="st4")
                nc.sync.dma_start(out=st_sb[:, 0:ro], in_=sv)
                xo_sb = xbp.tile([P, 4, D], f32, tag="xo")
                nc.vector.tensor_tensor(
                    out=xo_sb[:, 0:ro, :],
                    in0=xt_sb[:, 0:ro, :],
                    in1=st_sb[:, 0:ro, None].to_broadcast((P, ro, D)),
                    op=mybir.AluOpType.mult,
                )
                xpv = xp_d[a:a + rows, :].rearrange("(o p) d -> p o d", p=P)
                nc.sync.dma_start(out=xpv, in_=xo_sb[:, 0:ro, :])

            nfull = N // GR
            for t in range(nfull):
                scale_rows(t * GR, 4)
            a = nfull * GR
            while N - a >= P:
                ro = min(4, (N - a) // P)
                scale_rows(a, ro)
                a += ro * P
            tail = N - a
            if tail:
                xt_sb = xbp.tile([P, 4, D], f32, tag="xt")
                nc.sync.dma_start(out=xt_sb[0:tail, 0, :], in_=x_d[a:N, :])
                st_sb = xbp.tile([P, 4], f32, tag="st4")
                nc.sync.dma_start(out=st_sb[0:tail, 0:1], in_=s_allg[a:N, :])
                xo_sb = xbp.tile([P, 4, D], f32, tag="xo")
                nc.vector.tensor_tensor(
                    out=xo_sb[0:tail, 0, :],
                    in0=xt_sb[0:tail, 0, :],
                    in1=st_sb[0:tail, 0:1].to_broadcast((tail, D)),
                    op=mybir.AluOpType.mult,
                )
                nc.sync.dma_start(out=xp_d[a:N, :], in_=xo_sb[0:tail, 0, :])

            # ---- pass 2: gather + one-hot matmul aggregation ----
            first2 = {b: first[b][0] for b in first}
            mean_ps = pmp.tile([1, D], f32, tag="mean")
            xp_lo = xp_d[0:SPLIT, :]
            xp_hi = xp_d[SPLIT:N, :]
            for ci, (j0, nt, seg) in enumerate(chunks):
                xg = xgp.tile([P, CHUNK, D], f32, tag="xg")
                nc.gpsimd.dma_gather(
                    xg[:, 0:nt, :],
                    xp_lo if seg == 0 else xp_hi,
                    idx_sb[:, j0 * IDXW:(j0 + nt) * IDXW],
                    nt * P,
                    nt * P,
                    D,
                )
                if debug and ci == 0:
                    nc.sync.dma_start(out=dbg_xg[:], in_=xg[:])
                oh = ohp.tile([P, CHUNK, P], f32, tag="oh2")
                nc.vector.tensor_tensor(
                    out=oh[:, 0:nt, :],
                    in0=cl_sb[:, j0:j0 + nt, None].to_broadcast((P, nt, P)),
                    in1=iota_sb[:, None, :].to_broadcast((P, nt, P)),
                    op=mybir.AluOpType.is_equal,
                )
                for kk in range(nt):
                    j = j0 + kk
                    b = blk_of[j]
                    if j == first2[b]:
                        y_ps = pyp.tile([P, D], f32, tag="y")
                        first2[b] = (j, y_ps)
                    else:
                        y_ps = first2[b][1]
                    nc.tensor.matmul(
                        out=y_ps[:],
                        lhsT=oh[:, kk, :],
                        rhs=xg[:, kk, :],
                        start=(first2[b][0] == j),
                        stop=(last[b] == j),
                    )
                    if last[b] != j:
                        continue
                    # ---- block epilogue ----
                    y_sb = epp.tile([P, D], f32, tag="ysb")
                    nc.vector.tensor_copy(out=y_sb[:], in_=y_ps[:])
                    if debug and b == 0:
                        nc.sync.dma_start(out=dbg_y[:], in_=y_sb[:])
                    z_ps = pzp.tile([P, D], f32, tag="z")
                    for h in range(2):
                        yt_ps = ptp.tile([P, P], f32, tag="yt")
                        nc.tensor.transpose(
                            out=yt_ps[:],
                            in_=y_sb[:, h * P:(h + 1) * P],
                            identity=ident[:],
                        )
                        yt_sb = epp.tile([P, P], f32, tag="ytsb")
                        nc.vector.tensor_copy(out=yt_sb[:], in_=yt_ps[:])
                        nc.tensor.matmul(
                            out=z_ps[:],
                            lhsT=yt_sb[:],
                            rhs=(W0_sb if h == 0 else W1_sb)[:],
                            start=(h == 0),
                            stop=False,
                        )
                    nc.tensor.matmul(
                        out=z_ps[:],
                        lhsT=ones_r[:],
                        rhs=bc_sb[:],
                        start=False,
                        stop=True,
                    )
                    zs = epp.tile([P, D], f32, tag="zs")
                    nc.vector.tensor_scalar_mul(zs[:], z_ps[:], s_all[:, b:b + 1])
                    sig = epp.tile([P, D], f32, tag="sig")
                    nc.scalar.activation(
                        sig[:], zs[:], mybir.ActivationFunctionType.Sigmoid,
                    )
                    rows = LAST_ROWS if b == B - 1 else P
                    nc.tensor.matmul(
                        out=mean_ps[:],
                        lhsT=ones_c[0:rows, :],
                        rhs=sig[0:rows, :],
                        start=(b == 0),
                        stop=(b == B - 1),
                    )

            # ---- final: mean, all-reduce, linear, sigmoid ----
            mpart = epp.tile([1, D], f32, tag="mpart")
            nc.scalar.activation(
                mpart[:], mean_ps[:], mybir.ActivationFunctionType.Copy,
                scale=1.0 / N,
            )
            nc.sync.dma_start(out=mr_in[:], in_=mpart[:])
            if debug:
                nc.sync.dma_start(out=dbg_mp[:], in_=mpart[:])
            nc.gpsimd.collective_compute(
                "AllReduce",
                mybir.AluOpType.add,
                replica_groups=[list(range(C))],
                ins=[mr_in[:]],
                outs=[mr_out[:]],
            )
            msum = epp.tile([1, D], f32, tag="msum")
            nc.sync.dma_start(out=msum[:], in_=mr_out[:])
            if debug:
                nc.sync.dma_start(out=dbg_ms[:], in_=msum[:])
            prod = epp.tile([1, D], f32, tag="prod")
            nc.vector.tensor_tensor(
                out=prod[:], in0=msum[:], in1=wl_sb[:], op=mybir.AluOpType.mult,
            )
            dot = epp.tile([1, 1], f32, tag="dot")
            nc.vector.tensor_reduce(
                out=dot[:], in_=prod[:], axis=mybir.AxisListType.X,
                op=mybir.AluOpType.add,
            )
            zf = epp.tile([1, 1], f32, tag="zf")
            nc.vector.tensor_add(zf[:], dot[:], bl_sb[:])
            res = epp.tile([1, 1], f32, tag="res")
            nc.scalar.activation(
                res[:], zf[:], mybir.ActivationFunctionType.Sigmoid,
            )
            nc.sync.dma_start(out=out_d[:], in_=res[:])

    nc.compile()
    return nc


def _make_in_maps(inputs, idx_p, cl_p):
    iota = np.broadcast_to(np.arange(P, dtype=np.float32), (P, P)).copy()
    common = {
        "x": np.ascontiguousarray(np.asarray(inputs["x"], np.float32)),
        "W": np.asarray(inputs["W"], np.float32),
        "bconv": np.asarray(inputs["b_conv"], np.float32).reshape(1, D),
        "wlin": np.asarray(inputs["w_lin"], np.float32).reshape(1, D),
        "blin": np.asarray(inputs["b_lin"], np.float32).reshape(1, 1),
        "iota": iota,
    }
    return [
        {**common, "idx": idx_p[ci], "cl": cl_p[ci]}
        for ci in range(C)
    ]


def kernel(x, edge_index, W, b_conv, w_lin, b_lin):
    from concourse.bass_utils import run_bass_kernel_spmd

    idx_p, cl_p, T_seg, T = _prep_host(edge_index)

    key = tuple(tuple(t) for t in T_seg)
    if key not in _cache:
        _cache.clear()
        _cache[key] = _build(T_seg, T)
    nc = _cache[key]

    in_maps = _make_in_maps(
        {"x": x, "W": W, "b_conv": b_conv, "w_lin": w_lin, "b_lin": b_lin},
        idx_p, cl_p,
    )
    res = run_bass_kernel_spmd(nc, in_maps, list(range(C)))
    return res.results[0]["out"].reshape(1).astype(np.float32)



# revision 7
# speedup vs baseline: 1.7995x; 1.7995x over previous
"""GCN discriminator kernel for 8 Trainium2 NeuronCores.

Strategy (edge-parallel by destination, V2): all index-derived math is
done on host: degrees, s = 1/sqrt(deg), and the per-edge symmetric norm
val_e = s[src]*s[dst] (self-loops appended as ordinary edges).  Edges
are bucketed by (core = dst // 6250, dst block of 128, src lo/hi int16
segment) and padded to whole 128-edge tiles (pad: idx=0, col=128,
val=0).  x is cast to bf16 on host and gathered directly.

Each core then runs a single pass over its ~941 tiles:
  - dma_gather pulls 8 tiles (1024 rows) of bf16 source rows per
    call, spanning dst blocks, directly from the bf16 x table.
  - per tile, one fused DVE op builds the *valued* one-hot
    oh[p,j] = (iota[p,j] == col[p]) * val[p]  (bf16),
    and one bf16 matmul accumulates oh^T @ xrows into the dst block's
    PSUM [128,256] — this is the final normalized aggregation.
  - per dst block epilogue: y -> bf16, transpose, z = y@W + b_conv,
    sigmoid, and a ones-vector matmul accumulates the column sum of
    sigmoid rows for the mean.
Blocks are processed in groups of 4 (lo tiles of the group, then hi
tiles) so gather calls span blocks while only ~8 block PSUMs are live.
Final: AllReduce the [1,256] partial sums, mean, dot w_lin, sigmoid.
"""

import sys

for _p in ("/opt/trn_rl_repo", "/root/.axon_site/_ro/trn_rl_repo"):
    if _p not in sys.path:
        sys.path.insert(0, _p)

import numpy as np

N = 50000
E = 800000
D = 256
C = 8            # cores
NS = N // C      # dst rows per core
P = 128
B = (NS + P - 1) // P          # dst blocks per core (49; last has 106 rows)
LAST_ROWS = NS - (B - 1) * P   # 106
SPLIT = 32768                  # int16 index reach of dma_gather
CHUNK = 8                      # tiles per dma_gather call
G = 4                          # dst blocks per group (bounds live PSUM tiles)
IDXW = P // 16                 # idx columns per tile (wrapped in 16 parts)

_cache = {}


def _schedule(T_seg):
    """Static tile order shared by host prep and program build.

    Blocks are grouped G at a time; within a group all lo-segment tiles
    come first (chunked runs spanning blocks), then all hi tiles.
    Returns (tiles, first, last, chunks, tile_index) where
    tiles[j] = (block, seg), chunks = (j0, ntiles, seg), and
    tile_index[(b, seg)] = first global tile index of that run.
    """
    tiles = []
    tile_index = {}
    for g0 in range(0, B, G):
        for seg in range(2):
            for b in range(g0, min(g0 + G, B)):
                tile_index[(b, seg)] = len(tiles)
                tiles += [(b, seg)] * T_seg[b][seg]
    chunks = []
    j = 0
    T = len(tiles)
    while j < T:
        seg = tiles[j][1]
        nt = 1
        while nt < CHUNK and j + nt < T and tiles[j + nt][1] == seg:
            nt += 1
        chunks.append((j, nt, seg))
        j += nt
    first = {}
    last = {}
    for jj, (b, _) in enumerate(tiles):
        first.setdefault(b, jj)
        last[b] = jj
    return tiles, first, last, chunks, tile_index


def _prep_host(edge_index):
    """Host-side index math + bucketing.

    Computes deg/s/per-edge norms, buckets edges (+self loops) by
    (core, dst block, lo/hi source segment), pads every (block,seg) run
    to the max whole-tile count across cores.  Pad slots: idx=0,
    col=128 (one-hot row all-zero), val=0.
    """
    import ml_dtypes

    r_all = np.concatenate([np.asarray(edge_index[0], np.int64),
                            np.arange(N, dtype=np.int64)])
    c_all = np.concatenate([np.asarray(edge_index[1], np.int64),
                            np.arange(N, dtype=np.int64)])
    deg = np.bincount(c_all, minlength=N).astype(np.float64)
    s = 1.0 / np.sqrt(deg)                      # every node has a self-loop
    val = (s[r_all] * s[c_all]).astype(np.float32)

    core = c_all // NS
    rem = c_all % NS
    cl = (rem % P).astype(np.float32)
    seg = (r_all >= SPLIT).astype(np.int64)
    bucket = (core * B + rem // P) * 2 + seg
    order = np.argsort(bucket, kind="stable")
    counts = np.bincount(bucket, minlength=C * B * 2).reshape(C, B, 2)
    T_seg = (-(-counts // P)).max(axis=0)       # [B, 2]

    tiles, first, last, chunks, tile_index = _schedule(
        [[int(v) for v in x] for x in T_seg])
    T = len(tiles)

    starts = np.zeros(C * B * 2 + 1, np.int64)
    np.cumsum(counts.reshape(-1), out=starts[1:])

    r_t = np.zeros((C, T * P), np.int64)            # pad idx -> row 0
    cl_p = np.full((C, T * P), P, np.float32)       # pad col -> 128
    vl_p = np.zeros((C, T * P), np.float32)         # pad val -> 0
    rs = r_all[order]
    cs = cl[order]
    vs = val[order]
    for ci in range(C):
        for b in range(B):
            for sg in range(2):
                cnt = int(counts[ci, b, sg])
                s0 = int(starts[(ci * B + b) * 2 + sg])
                d0 = tile_index[(b, sg)] * P
                r_t[ci, d0:d0 + cnt] = rs[s0:s0 + cnt] - (SPLIT if sg else 0)
                cl_p[ci, d0:d0 + cnt] = cs[s0:s0 + cnt]
                vl_p[ci, d0:d0 + cnt] = vs[s0:s0 + cnt]

    # [C, T*P] -> [C, P, T]: tile j is column j, edge slot q is partition q
    cl_p = np.ascontiguousarray(cl_p.reshape(C, T, P).transpose(0, 2, 1))
    vl_p = np.ascontiguousarray(vl_p.reshape(C, T, P).transpose(0, 2, 1))
    # idx param: element i of a call at [i % 16, col0 + i // 16],
    # replicated 8x down the partitions.  Calls are tile-aligned, so
    # tile j owns idx columns [j*IDXW, (j+1)*IDXW).
    idx16 = r_t.reshape(C, T * IDXW, 16).transpose(0, 2, 1).astype(np.int16)
    idx_p = np.ascontiguousarray(np.tile(idx16, (1, 8, 1)))  # [C, 128, T*8]

    return idx_p, cl_p, vl_p, [[int(v) for v in x] for x in T_seg], T


def _build(T_seg, T):
    from concourse import bass, bacc, mybir
    import concourse.tile as tile
    from concourse.masks import make_identity

    f32 = mybir.dt.float32
    bf16 = mybir.dt.bfloat16
    i16 = mybir.dt.int16

    nc = bacc.Bacc(
        "TRN2",
        target_bir_lowering=False,
        debug=False,
        num_devices=C,
    )

    xb_d = nc.declare_dram_parameter("xb", [N, D], bf16, isOutput=False)
    idx_d = nc.declare_dram_parameter("idx", [P, T * IDXW], i16, isOutput=False)
    cl_d = nc.declare_dram_parameter("cl", [P, T], f32, isOutput=False)
    vl_d = nc.declare_dram_parameter("vl", [P, T], f32, isOutput=False)
    W_d = nc.declare_dram_parameter("W", [D, D], bf16, isOutput=False)
    bc_d = nc.declare_dram_parameter("bconv", [1, D], bf16, isOutput=False)
    wl_d = nc.declare_dram_parameter("wlin", [1, D], f32, isOutput=False)
    bl_d = nc.declare_dram_parameter("blin", [1, 1], f32, isOutput=False)
    io_d = nc.declare_dram_parameter("iota", [P, P], bf16, isOutput=False)
    out_d = nc.declare_dram_parameter("out", [1, 1], f32, isOutput=True)

    mr_in = nc.dram_tensor("mr_in", [1, D], f32)
    mr_out = nc.dram_tensor("mr_out", [1, D], f32, addr_space="Shared")

    tiles, first, last, chunks, _ = _schedule(T_seg)

    with tile.TileContext(nc) as tc:
        with tc.tile_pool(name="static", bufs=1) as st, \
             tc.tile_pool(name="oh", bufs=6) as ohp, \
             tc.tile_pool(name="xg", bufs=3) as xgp, \
             tc.tile_pool(name="ep", bufs=2) as epp, \
             tc.tile_pool(name="py", bufs=G + 1, space="PSUM") as pyp, \
             tc.tile_pool(name="pt", bufs=1, space="PSUM") as ptp, \
             tc.tile_pool(name="pz", bufs=1, space="PSUM") as pzp, \
             tc.tile_pool(name="pm", bufs=1, space="PSUM") as pmp:

            # ---- static loads ----
            idx_sb = st.tile([P, T * IDXW], i16, tag="idx")
            cl_sb = st.tile([P, T], f32, tag="cl")
            vl_sb = st.tile([P, T], f32, tag="vl")
            nc.sync.dma_start(out=idx_sb[:], in_=idx_d[:])
            nc.sync.dma_start(out=cl_sb[:], in_=cl_d[:])
            nc.sync.dma_start(out=vl_sb[:], in_=vl_d[:])
            iota_sb = st.tile([P, P], bf16, tag="iota")
            nc.sync.dma_start(out=iota_sb[:], in_=io_d[:])
            W0_sb = st.tile([P, D], bf16, tag="w0")
            W1_sb = st.tile([P, D], bf16, tag="w1")
            nc.sync.dma_start(out=W0_sb[:], in_=W_d[0:P, :])
            nc.sync.dma_start(out=W1_sb[:], in_=W_d[P:D, :])
            bc_sb = st.tile([1, D], bf16, tag="bc")
            nc.sync.dma_start(out=bc_sb[:], in_=bc_d[:])
            wl_sb = st.tile([1, D], f32, tag="wl")
            nc.sync.dma_start(out=wl_sb[:], in_=wl_d[:])
            bl_sb = st.tile([1, 1], f32, tag="bl")
            nc.sync.dma_start(out=bl_sb[:], in_=bl_d[:])
            ident = st.tile([P, P], bf16, tag="ident")
            make_identity(nc, ident[:])
            ones_c = st.tile([P, 1], bf16, tag="onesc")
            nc.vector.memset(ones_c[:], 1.0)
            ones_r = st.tile([1, P], bf16, tag="onesr")
            nc.vector.memset(ones_r[:], 1.0)

            mean_ps = pmp.tile([1, D], f32, tag="mean")
            xb_lo = xb_d[0:SPLIT, :]
            xb_hi = xb_d[SPLIT:N, :]
            y_of = dict(first)          # block -> (first tile j, psum tile)

            for (j0, nt, seg) in chunks:
                xg = xgp.tile([P, CHUNK, D], bf16, tag="xg")
                nc.gpsimd.dma_gather(
                    xg[:, 0:nt, :],
                    xb_lo if seg == 0 else xb_hi,
                    idx_sb[:, j0 * IDXW:(j0 + nt) * IDXW],
                    nt * P,
                    nt * P,
                    D,
                )
                for kk in range(nt):
                    j = j0 + kk
                    b = tiles[j][0]
                    oh = ohp.tile([P, P], bf16, tag="oh")
                    nc.vector.tensor_scalar(
                        out=oh[:],
                        in0=iota_sb[:],
                        scalar1=cl_sb[:, j:j + 1],
                        scalar2=vl_sb[:, j:j + 1],
                        op0=mybir.AluOpType.is_equal,
                        op1=mybir.AluOpType.mult,
                    )
                    if j == y_of[b]:
                        y_ps = pyp.tile([P, D], f32, tag="y")
                        y_of[b] = (j, y_ps)
                    else:
                        y_ps = y_of[b][1]
                    nc.tensor.matmul(
                        out=y_ps[:],
                        lhsT=oh[:],
                        rhs=xg[:, kk, :],
                        start=(y_of[b][0] == j),
                        stop=(last[b] == j),
                    )
                    if last[b] != j:
                        continue
                    # ---- block epilogue ----
                    y_sb = epp.tile([P, D], bf16, tag="ysb")
                    nc.scalar.activation(
                        y_sb[:], y_ps[:], mybir.ActivationFunctionType.Copy,
                    )
                    z_ps = pzp.tile([P, D], f32, tag="z")
                    for h in range(2):
                        yt_ps = ptp.tile([P, P], bf16, tag="yt")
                        nc.tensor.transpose(
                            out=yt_ps[:],
                            in_=y_sb[:, h * P:(h + 1) * P],
                            identity=ident[:],
                        )
                        yt_sb = epp.tile([P, P], bf16, tag="ytsb")
                        nc.vector.tensor_copy(out=yt_sb[:], in_=yt_ps[:])
                        nc.tensor.matmul(
                            out=z_ps[:],
                            lhsT=yt_sb[:],
                            rhs=(W0_sb if h == 0 else W1_sb)[:],
                            start=(h == 0),
                            stop=False,
                        )
                    nc.tensor.matmul(
                        out=z_ps[:],
                        lhsT=ones_r[:],
                        rhs=bc_sb[:],
                        start=False,
                        stop=True,
                    )
                    sig = epp.tile([P, D], bf16, tag="sig")
                    nc.scalar.activation(
                        sig[:], z_ps[:], mybir.ActivationFunctionType.Sigmoid,
                    )
                    rows = LAST_ROWS if b == B - 1 else P
                    nc.tensor.matmul(
                        out=mean_ps[:],
                        lhsT=ones_c[0:rows, :],
                        rhs=sig[0:rows, :],
                        start=(b == 0),
                        stop=(b == B - 1),
                    )

            # ---- final: mean, all-reduce, linear, sigmoid ----
            mpart = epp.tile([1, D], f32, tag="mpart")
            nc.scalar.activation(
                mpart[:], mean_ps[:], mybir.ActivationFunctionType.Copy,
                scale=1.0 / N,
            )
            nc.sync.dma_start(out=mr_in[:], in_=mpart[:])
            nc.gpsimd.collective_compute(
                "AllReduce",
                mybir.AluOpType.add,
                replica_groups=[list(range(C))],
                ins=[mr_in[:]],
                outs=[mr_out[:]],
            )
            msum = epp.tile([1, D], f32, tag="msum")
            nc.sync.dma_start(out=msum[:], in_=mr_out[:])
            prod = epp.tile([1, D], f32, tag="prod")
            nc.vector.tensor_tensor(
                out=prod[:], in0=msum[:], in1=wl_sb[:], op=mybir.AluOpType.mult,
            )
            dot = epp.tile([1, 1], f32, tag="dot")
            nc.vector.tensor_reduce(
                out=dot[:], in_=prod[:], axis=mybir.AxisListType.X,
                op=mybir.AluOpType.add,
            )
            zf = epp.tile([1, 1], f32, tag="zf")
            nc.vector.tensor_add(zf[:], dot[:], bl_sb[:])
            res = epp.tile([1, 1], f32, tag="res")
            nc.scalar.activation(
                res[:], zf[:], mybir.ActivationFunctionType.Sigmoid,
            )
            nc.sync.dma_start(out=out_d[:], in_=res[:])

    nc.compile()
    return nc


def _make_in_maps(inputs, idx_p, cl_p, vl_p):
    import ml_dtypes

    bf16 = ml_dtypes.bfloat16
    iota = np.broadcast_to(np.arange(P, dtype=np.float32), (P, P)).astype(bf16)
    common = {
        "xb": np.ascontiguousarray(np.asarray(inputs["x"], np.float32)).astype(bf16),
        "W": np.asarray(inputs["W"], np.float32).astype(bf16),
        "bconv": np.asarray(inputs["b_conv"], np.float32).reshape(1, D).astype(bf16),
        "wlin": np.asarray(inputs["w_lin"], np.float32).reshape(1, D),
        "blin": np.asarray(inputs["b_lin"], np.float32).reshape(1, 1),
        "iota": np.ascontiguousarray(iota),
    }
    return [
        {**common, "idx": idx_p[ci], "cl": cl_p[ci], "vl": vl_p[ci]}
        for ci in range(C)
    ]


def kernel(x, edge_index, W, b_conv, w_lin, b_lin):
    from concourse.bass_utils import run_bass_kernel_spmd

    idx_p, cl_p, vl_p, T_seg, T = _prep_host(edge_index)

    key = tuple(tuple(t) for t in T_seg)
    if key not in _cache:
        _cache.clear()
        _cache[key] = _build(T_seg, T)
    nc = _cache[key]

    in_maps = _make_in_maps(
        {"x": x, "W": W, "b_conv": b_conv, "w_lin": w_lin, "b_lin": b_lin},
        idx_p, cl_p, vl_p,
    )
    res = run_bass_kernel_spmd(nc, in_maps, list(range(C)))
    return res.results[0]["out"].reshape(1).astype(np.float32)
